# revision 14
# baseline (speedup 1.0000x reference)
"""Trainium2 Bass kernel for nn_MixingBlock (local-window attention + MLP).

Sharding: 8 cores = (batch 0..3) x (token half 0..1); each core computes
1024 output tokens of one batch element on a zero-padded 22-image-row slab
(T=1408 tokens); no collectives (see kernel_baseline.py docstring).

v3 structure:
- additive mask (0 / -30000, bf16) folded into the score PSUM via
  identity-weight matmuls (one identity LDWEIGHTS per head-pair group);
- AV matmuls use V as the stationary operand (head-pairs, M=65: a-dims,
  b-dims, shared ones column) with the exp'd scores moving (N=256):
  128 MMs + 128 small LDWEIGHTS instead of 256 + 256 full-width ones, and
  the output lands directly in proj-ready (channels, queries) layout with
  softmax denominators as aligned extra rows -> no PE transposes after
  attention;
- denominators: 4 aligned row-copies into a memset-1.0 tile, one
  reciprocal, and an E128 selector matmul broadcasting each recip row
  across its 32-row head block; one tensor_tensor multiply per head slice
  normalizes while copying PSUM->SBUF bf16 (engine partition bases must
  be 32-aligned, DVE has no divide op);
- residual adds (x+proj, z1g+mlp2) folded into PSUM via identity matmuls;
- ln1_g applied once to z1 (w1 kept raw); LN scale/add tail ops run on
  the otherwise-idle gpsimd engine (SBUF-only operands);
- K-projection PSUM->SBUF copies on vector, Q on scalar; Q projected only
  over the 1024 query tokens;
- inputs packed host-side into one multi-slice SBUF tile per tensor so
  each needs a single DMA; xres shipped as bf16; DMAs spread across the
  sync/gpsimd/scalar queues with bulk weights deferred past the first MM.
"""

import contextlib
import sys
import types

import ml_dtypes
import numpy as np

import concourse.bass as bass
import concourse.mybir as mybir
import concourse.tile as tile

# ---------------------------------------------------------------------------
# axon NTFF profile hook (lets run_bass_kernel_spmd(trace=True) work here)
# ---------------------------------------------------------------------------
if "antenv.axon_hooks" not in sys.modules:
    try:
        import antenv  # noqa: F401

        _hookmod = types.ModuleType("antenv.axon_hooks")
        _hookmod._hook = None
        _hookmod.set_axon_ntff_profile_hook = lambda h: setattr(_hookmod, "_hook", h)
        _hookmod.get_axon_ntff_profile_hook = lambda: _hookmod._hook
        sys.modules["antenv.axon_hooks"] = _hookmod
        try:
            from trn_agent_boot.trn_boot import _ntff_profile_via_ctypes

            _hookmod.set_axon_ntff_profile_hook(
                _ntff_profile_via_ctypes("/opt/axon/libaxon_pjrt.so")
            )
        except Exception:
            pass
    except Exception:
        pass

from concourse.bass_utils import run_bass_kernel_spmd  # noqa: E402
from concourse.masks import make_identity  # noqa: E402
from concourse.tile_rust import add_dep_helper  # noqa: E402

F32 = mybir.dt.float32
BF16 = mybir.dt.bfloat16
AF = mybir.ActivationFunctionType
ALU = mybir.AluOpType

# Problem constants
H_IMG, W_IMG = 32, 64
N = H_IMG * W_IMG  # 2048
C = 256
NH = 8
HD = 32
HIDDEN = 1024
SCALE = HD**-0.5
B = 4
LN_EPS = 1e-5

# Sharding constants
PAD_ROWS = 3
SLAB_ROWS = 16 + 2 * PAD_ROWS  # 22
T = SLAB_ROWS * W_IMG  # 1408
NQ = 1024
NQT = 8
KW = 512  # key window tokens per query tile
Q0 = PAD_ROWS * W_IMG  # 192: first query token within slab
NEG = -30000.0  # additive mask value for invalid keys

MAX_WAITS = 1


def _split_excess_waits(nc):
    """walrus accepts only MAX_WAITS sem-waits per instruction; move excess
    onto same-engine nops inserted immediately before the instruction."""
    for f in nc.m.functions:
        for bb in f.blocks:
            i = 0
            while i < len(bb.instructions):
                ins = bb.instructions[i]
                si = ins.sync_info
                if si is not None and si.on_wait and len(si.on_wait) > MAX_WAITS:
                    waits = list(si.on_wait)
                    extra, keep = waits[:-MAX_WAITS], waits[-MAX_WAITS:]
                    ins.sync_info = mybir.SyncInfo(
                        on_wait=keep, on_update=list(si.on_update)
                    )
                    nops = []
                    for j in range(0, len(extra), MAX_WAITS):
                        nop = nc.engines[ins.engine].nop().ins
                        cur = nc.cur_bb.bb
                        assert cur.instructions[-1] is nop
                        cur.instructions.pop()
                        nop.sync_info = mybir.SyncInfo(
                            on_wait=extra[j : j + MAX_WAITS], on_update=[]
                        )
                        nops.append(nop)
                    bb.instructions[i:i] = nops
                    i += len(nops)
                i += 1


def _build_nc():
    nc = bass.Bass("TRN2", target_bir_lowering=False, num_devices=8)

    d = {}
    d["xT"] = nc.dram_tensor("xT", [128, 2, T], BF16, kind="ExternalInput")
    d["xres"] = nc.dram_tensor("xres", [128, 8, C], BF16, kind="ExternalInput")
    d["wqk"] = nc.dram_tensor("wqk", [128, 2, 512], BF16, kind="ExternalInput")
    d["qkb"] = nc.dram_tensor("qkb", [128, 4], F32, kind="ExternalInput")
    d["wv"] = nc.dram_tensor("wv", [128, 2, 260], BF16, kind="ExternalInput")
    d["wp"] = nc.dram_tensor("wp", [128, 2, C], BF16, kind="ExternalInput")
    d["w1"] = nc.dram_tensor("w1", [128, 2, HIDDEN], BF16, kind="ExternalInput")
    d["b1"] = nc.dram_tensor("b1", [128, 8], F32, kind="ExternalInput")
    d["w2"] = nc.dram_tensor("w2", [128, 8, C], BF16, kind="ExternalInput")
    d["rows"] = nc.dram_tensor("rows", [1, 3 * C], F32, kind="ExternalInput")
    d["bv2"] = nc.dram_tensor("bv2", [1, C], BF16, kind="ExternalInput")
    d["mask"] = nc.dram_tensor("mask", [128, NQT, KW], BF16, kind="ExternalInput")
    d["E128"] = nc.dram_tensor("E128", [128, 128], BF16, kind="ExternalInput")
    d["out"] = nc.dram_tensor("out", [NQ, C], F32, kind="ExternalOutput")

    with tile.TileContext(nc) as tc:
        _emit(nc, tc, d)

    _split_excess_waits(nc)
    return nc


def _emit(nc, tc, d):
    ctx = contextlib.ExitStack()
    with ctx:
        const = ctx.enter_context(tc.tile_pool(name="const", bufs=1))
        big = ctx.enter_context(tc.tile_pool(name="big", bufs=1))
        ps_s = ctx.enter_context(tc.tile_pool(name="ps_s", bufs=2, space="PSUM"))
        ps_av = ctx.enter_context(tc.tile_pool(name="ps_av", bufs=2, space="PSUM"))
        ps = ctx.enter_context(tc.tile_pool(name="ps", bufs=2, space="PSUM"))
        work = ctx.enter_context(tc.tile_pool(name="work", bufs=4))
        small = ctx.enter_context(tc.tile_pool(name="small", bufs=4))

        late_dmas = []
        # -------- inputs: one multi-slice tile + one DMA per tensor ------
        xT = const.tile([128, 2, T], BF16, tag="xT", name="xT")
        wqk = const.tile([128, 2, 512], BF16, tag="wqk", name="wqk")
        wv = const.tile([128, 2, 260], BF16, tag="wv", name="wv")
        wp = const.tile([128, 2, C], BF16, tag="wp", name="wp")
        w1 = const.tile([128, 2, HIDDEN], BF16, tag="w1", name="w1s")
        qkb = const.tile([128, 4], F32, tag="qkb", name="qkb")
        b1 = const.tile([128, 8], F32, tag="b1", name="b1s")
        w2 = const.tile([128, 8, C], BF16, tag="w2", name="w2s")
        xres = const.tile([128, 8, C], BF16, tag="xres", name="xres")
        mask = const.tile([128, NQT, KW], BF16, tag="mask", name="mask")
        E128 = const.tile([128, 128], BF16, tag="E128", name="E128")
        denB = [const.tile([128, 128], F32, tag=f"denB{g}", name=f"denB{g}") for g in range(2)]

        # early DMAs: sync feeds the QK projection; gpsimd feeds masks
        # (needed from qt=0), wv and E128
        nc.sync.dma_start(out=xT[:], in_=d["xT"][:, :, :])
        nc.sync.dma_start(out=wqk[:], in_=d["wqk"][:, :, :])
        nc.sync.dma_start(out=qkb[:], in_=d["qkb"][:, :])
        nc.gpsimd.dma_start(out=wv[:], in_=d["wv"][:, :, :])
        nc.gpsimd.dma_start(out=mask[:], in_=d["mask"][:, :, :])
        nc.gpsimd.dma_start(out=E128[:], in_=d["E128"][:])
        for g in range(2):
            nc.vector.memset(denB[g][:], 1.0)
        # late DMAs (deferred until the first matmul has issued)
        late_dmas.append(nc.scalar.dma_start(out=wp[:], in_=d["wp"][:, :, :]).ins)
        late_dmas.append(nc.sync.dma_start(out=xres[:], in_=d["xres"][:, :, :]).ins)
        late_dmas.append(nc.sync.dma_start(out=w1[:], in_=d["w1"][:, :, :]).ins)
        late_dmas.append(nc.sync.dma_start(out=b1[:], in_=d["b1"][:, :]).ins)
        late_dmas.append(nc.sync.dma_start(out=w2[:], in_=d["w2"][:, :, :]).ins)
        bv2 = const.tile([1, C], BF16)
        late_dmas.append(nc.sync.dma_start(out=bv2[:], in_=d["bv2"][:]).ins)
        g1b = const.tile([128, C], F32)
        g2b = const.tile([128, C], F32)
        b2lb = const.tile([128, C], F32)
        nc.scalar.dma_start(
            out=g1b[:], in_=d["rows"][0:1, 0:C].to_broadcast((128, C))
        )
        nc.scalar.dma_start(
            out=g2b[:], in_=d["rows"][0:1, C : 2 * C].to_broadcast((128, C))
        )
        nc.scalar.dma_start(
            out=b2lb[:], in_=d["rows"][0:1, 2 * C : 3 * C].to_broadcast((128, C))
        )

        eps_t = const.tile([128, 1], F32)
        nc.vector.memset(eps_t[:], LN_EPS)
        ones_col = const.tile([1, 128], BF16)
        nc.vector.memset(ones_col[:], 1.0)
        ident = const.tile([128, 128], BF16)
        make_identity(nc, ident)

        # ---------------- phase 1: Q^T (scaled) and K^T ----------------
        # m pieces 0,1 = Q (only query tokens needed), 2,3 = K (full slab)
        qkT = [big.tile([128, T], BF16, tag=f"qkT{m}", name=f"qkT{m}") for m in range(4)]
        first_mm = [None]

        def qk_piece(m, lo, hi):
            for off in range(lo, hi, 512):
                w = min(512, hi - off)
                p = ps_s.tile([128, 2, KW], F32, tag="s_ps", name="p_qk")
                for cc in range(2):
                    mm0 = nc.tensor.matmul(
                        p[:, 0, :w],
                        wqk[:, cc, 128 * m : 128 * (m + 1)],
                        xT[:, cc, off : off + w],
                        start=(cc == 0),
                        stop=(cc == 1),
                    )
                    if first_mm[0] is None:
                        first_mm[0] = mm0
                        for dma in late_dmas:
                            add_dep_helper(dma, mm0.ins, sync=True,
                                           reason="defer bulk input DMA")
                if m < 2:
                    nc.scalar.activation(
                        out=qkT[m][:, off : off + w],
                        in_=p[:, 0, :w],
                        func=AF.Identity,
                        bias=qkb[:, m : m + 1],
                        scale=1.0,
                    )
                else:
                    nc.vector.tensor_scalar_add(
                        out=qkT[m][:, off : off + w],
                        in0=p[:, 0, :w],
                        scalar1=qkb[:, m : m + 1],
                    )

        for m in range(2):
            qk_piece(m, Q0, Q0 + NQ)
        for m in range(2, 4):
            qk_piece(m, 0, T)

        # ---------------- phase 2: V (token-major, ones columns) ----------
        vt = [big.tile([128, 260], BF16, tag=f"vt{i}", name=f"vt{i}") for i in range(T // 128)]
        for i in range(T // 128):
            p = ps_s.tile([128, 260], F32, tag="s_ps", name="p_v")
            for cc in range(2):
                nc.tensor.matmul(
                    p[:, :260],
                    xT[:, cc, 128 * i : 128 * (i + 1)],
                    wv[:, cc, :],
                    start=(cc == 0),
                    stop=(cc == 1),
                )
            nc.vector.tensor_copy(vt[i][:], p[:, :260])
            nc.gpsimd.memset(
                vt[i][:].rearrange("p (g e) -> p g e", g=4)[:, :, 64:65], 1.0
            )

        # ---------------- phase 3: attention ----------------
        attnT = [
            [
                big.tile([128, 128], BF16, tag=f"attnT{j}_{q}", name=f"attnT{j}_{q}")
                for q in range(NQT)
            ]
            for j in range(2)
        ]
        for qt in range(NQT):
            kw0 = 128 * qt
            for g in range(2):  # head-pair groups: pairs (2g, 2g+1)
                # av tile: [partition, pair-in-group, hi, query]
                av = ps_av.tile([128, 2, 2, 128], F32, tag="av", name=f"av{qt}_{g}")
                pss = [
                    ps_s.tile([128, 2, KW], F32, tag="s_ps", name=f"p_s{qt}_{2*g+pp}")
                    for pp in range(2)
                ]
                # additive mask first (one ident LDW for all 4 regions)
                for pp in range(2):
                    for hi in range(2):
                        nc.tensor.matmul(
                            pss[pp][:, hi, :], ident[:], mask[:, qt, :],
                            start=True, stop=False,
                        )
                for pp in range(2):
                    p_s = pss[pp]
                    h0 = 2 * (2 * g + pp)
                    for c in range(4):
                        for hi in range(2):
                            h = h0 + hi
                            ktile, koff = 2 + h // 4, (32 * h) % 128
                            qtile, qoff = h // 4, (32 * h) % 128
                            nc.tensor.matmul(
                                p_s[:, hi, 128 * c : 128 * (c + 1)],
                                qkT[ktile][
                                    koff : koff + 32,
                                    kw0 + 128 * c : kw0 + 128 * (c + 1),
                                ],
                                qkT[qtile][
                                    qoff : qoff + 32,
                                    Q0 + 128 * qt : Q0 + 128 * (qt + 1),
                                ],
                                start=False,
                                stop=(c == 3),
                                tile_position=(koff, 0),
                            )
                    pT = work.tile([128, 2, KW], BF16, tag="pT", name=f"pT{qt}_{pp}")
                    nc.scalar.activation(
                        out=pT[:], in_=p_s[:], func=AF.Exp, bias=0.0, scale=1.0
                    )
                    # AV: V pair stationary (M=65: a-dims, b-dims, ones col)
                    pv = 2 * g + pp
                    for c in range(4):
                        nc.tensor.matmul(
                            av[0:65, pp, :, :],
                            vt[qt + c][:, 65 * pv : 65 * pv + 65],
                            pT[:, :, 128 * c : 128 * (c + 1)],
                            start=(c == 0),
                            stop=(c == 3),
                        )
                # denominators: ones-rows -> 32k rows of denB, recip, then
                # R = E128^T @ rec expands row 32k to rows [32k,32k+32)
                for pp in range(2):
                    for hi in range(2):
                        k = 2 * pp + hi
                        nc.vector.tensor_copy(
                            denB[g][32 * k : 32 * k + 1, :], av[64:65, pp, hi, :]
                        )
                recB = small.tile([128, 128], BF16, tag="rec", name=f"rec{qt}_{g}")
                with nc.allow_low_precision(reason="bf16 recip of softmax denom"):
                    nc.vector.reciprocal(recB[:], denB[g][:])
                Rp = ps.tile([128, 128], F32, tag="mm", name=f"Rp{qt}_{g}")
                nc.tensor.matmul(Rp[:, :], E128[:], recB[:], start=True, stop=True)
                R = work.tile([128, 128], BF16, tag="Rs", name=f"Rs{qt}_{g}")
                nc.vector.tensor_copy(R[:], Rp[:, :])
                # normalize + copy to attnT (tile g holds heads 4g..4g+3)
                aT = attnT[g][qt]
                for pp in range(2):
                    for hi in range(2):
                        k = 2 * pp + hi
                        nc.vector.tensor_tensor(
                            out=aT[64 * pp + 32 * hi : 64 * pp + 32 * hi + 32, :],
                            in0=av[32 * hi : 32 * hi + 32, pp, hi, :],
                            in1=R[32 * k : 32 * k + 32, :],
                            op=ALU.mult,
                        )

        # ---------------- phase 4: proj + residual + LN1 ----------------
        z1g = [big.tile([128, C], BF16, tag=f"z1g{i}", name=f"z1g_{i}") for i in range(8)]
        for t in range(8):
            p_p = ps.tile([128, C], F32, tag="mm", name="p_p")
            nc.tensor.matmul(p_p[:, :C], ident[:], xres[:, t, :], start=True, stop=False)
            for cc in range(2):
                nc.tensor.matmul(
                    p_p[:, :C],
                    attnT[cc][t][:],
                    wp[:, cc, :],
                    start=False,
                    stop=(cc == 1),
                )
            stats = small.tile([128, 6], F32, tag="stats")
            nc.vector.bn_stats(out=stats[:], in_=p_p[:, :C])
            mv = small.tile([128, 2], F32, tag="mv")
            nc.vector.bn_aggr(out=mv[:], in_=stats[:])
            lnv = small.tile([128, 1], F32, tag="lnv")
            nc.scalar.activation(
                out=lnv[:], in_=mv[:, 1:2], func=AF.Ln, bias=eps_t[:], scale=1.0
            )
            rstd = small.tile([128, 1], F32, tag="rstd")
            nc.scalar.activation(
                out=rstd[:], in_=lnv[:], func=AF.Exp, bias=0.0, scale=-0.5
            )
            zc = work.tile([128, C], F32, tag="zc")
            nc.vector.scalar_tensor_tensor(
                out=zc[:],
                in0=p_p[:, :C],
                scalar=mv[:, 0:1],
                in1=g1b[:],
                op0=ALU.subtract,
                op1=ALU.mult,
            )
            nc.gpsimd.tensor_scalar_mul(out=z1g[t][:], in0=zc[:], scalar1=rstd[:])

        # ---------------- phase 4b: z1g^T (PE transpose) ----------------
        z1T = [
            [
                big.tile([128, 512], BF16, tag=f"z1T{j}_{p}", name=f"z1T{j}_{p}")
                for p in range(2)
            ]
            for j in range(2)
        ]
        for t in range(8):
            for j in range(2):
                p_t = ps.tile([128, 128], BF16, tag="mm", name="p_t")
                nc.tensor.transpose(
                    p_t[:, :128], z1g[t][:, 128 * j : 128 * (j + 1)], ident[:]
                )
                nc.vector.tensor_copy(
                    z1T[j][t // 4][:, 128 * (t % 4) : 128 * (t % 4 + 1)],
                    p_t[:, :128],
                )

        # ---------------- phase 5: mlp1 + gelu (hidden-major) ------------
        hT = [
            [
                big.tile([128, 512], BF16, tag=f"hT{i}_{p}", name=f"hT{i}_{p}")
                for p in range(2)
            ]
            for i in range(8)
        ]
        for piece in range(2):
            for hc in range(8):
                p_h = ps_s.tile([128, 2, KW], F32, tag="s_ps", name="p_h")
                for cc in range(2):
                    nc.tensor.matmul(
                        p_h[:, 0, :],
                        w1[:, cc, 128 * hc : 128 * (hc + 1)],
                        z1T[cc][piece][:],
                        start=(cc == 0),
                        stop=(cc == 1),
                    )
                nc.scalar.activation(
                    out=hT[hc][piece][:],
                    in_=p_h[:, 0, :],
                    func=AF.Gelu,
                    bias=b1[:, hc : hc + 1],
                    scale=1.0,
                )

        # ---------------- phase 6: mlp2 + resid2 + LN2 + out -------------
        for t in range(8):
            p_m = ps.tile([128, C], F32, tag="mm", name="p_m")
            nc.tensor.matmul(p_m[:, :C], ones_col[:], bv2[:], start=True, stop=False)
            nc.tensor.matmul(p_m[:, :C], ident[:], z1g[t][:], start=False, stop=False)
            for hc in range(8):
                nc.tensor.matmul(
                    p_m[:, :C],
                    hT[hc][t // 4][:, 128 * (t % 4) : 128 * (t % 4 + 1)],
                    w2[:, hc, :],
                    start=False,
                    stop=(hc == 7),
                )
            stats = small.tile([128, 6], F32, tag="stats2")
            nc.vector.bn_stats(out=stats[:], in_=p_m[:, :C])
            mv = small.tile([128, 2], F32, tag="mv2")
            nc.vector.bn_aggr(out=mv[:], in_=stats[:])
            lnv = small.tile([128, 1], F32, tag="lnv2")
            nc.scalar.activation(
                out=lnv[:], in_=mv[:, 1:2], func=AF.Ln, bias=eps_t[:], scale=1.0
            )
            rstd = small.tile([128, 1], F32, tag="rstd2")
            nc.scalar.activation(
                out=rstd[:], in_=lnv[:], func=AF.Exp, bias=0.0, scale=-0.5
            )
            zc = work.tile([128, C], F32, tag="zc2")
            nc.vector.scalar_tensor_tensor(
                out=zc[:],
                in0=p_m[:, :C],
                scalar=mv[:, 0:1],
                in1=g2b[:],
                op0=ALU.subtract,
                op1=ALU.mult,
            )
            zz = work.tile([128, C], F32, tag="zz")
            nc.gpsimd.tensor_scalar_mul(out=zz[:], in0=zc[:], scalar1=rstd[:])
            o = work.tile([128, C], F32, tag="o")
            nc.gpsimd.tensor_add(o[:], zz[:], b2lb[:])
            nc.sync.dma_start(out=d["out"][128 * t : 128 * (t + 1), :], in_=o[:])


_NC_CACHE = None
_LAST_RESULT = None


def _get_nc():
    global _NC_CACHE
    if _NC_CACHE is None:
        _NC_CACHE = _build_nc()
    return _NC_CACHE


def _to_bf16(a):
    return np.ascontiguousarray(np.asarray(a, dtype=np.float32)).astype(
        ml_dtypes.bfloat16
    )


def _fold128(a):
    """[128*k, cols...] -> [128, k, cols...] (partition-major packing)."""
    a = np.asarray(a, dtype=np.float32)
    k = a.shape[0] // 128
    return np.ascontiguousarray(
        a.reshape(k, 128, *a.shape[1:]).transpose(1, 0, *range(2, a.ndim + 1))
    )


def _host_inputs(core, x, mask, qkv_w, qkv_b, proj_w, proj_b, ln1_g, ln1_b, w1,
                 b1, w2, b2, ln2_g, ln2_b):
    b = core // 2
    half = core % 2
    row0 = 16 * half - PAD_ROWS  # slab start image row (may be negative)
    S0 = row0 * W_IMG  # slab start token
    Q0g = 1024 * half  # first query token (global)

    xb = np.asarray(x[b], dtype=np.float32)  # [N, C]
    slab = np.zeros((T, C), np.float32)
    g_lo, g_hi = max(0, S0), min(N, S0 + T)
    slab[g_lo - S0 : g_hi - S0] = xb[g_lo:g_hi]

    wqk = np.concatenate([qkv_w[:C] * SCALE, qkv_w[C : 2 * C]], axis=0)  # [512,C]
    qkb = np.concatenate([qkv_b[:C] * SCALE, qkv_b[C : 2 * C]])  # [512]
    wv = qkv_w[2 * C :]  # [256, 256]
    vb = qkv_b[2 * C :]
    assert np.abs(vb).max() == 0.0, "nonzero v bias not folded"
    wv_pad = np.zeros((C, 260), np.float32)
    for p in range(4):
        wv_pad[:, 65 * p : 65 * p + 32] = wv[64 * p : 64 * p + 32].T
        wv_pad[:, 65 * p + 32 : 65 * p + 64] = wv[64 * p + 32 : 64 * p + 64].T

    b1f = b1 + w1 @ ln1_b  # fold ln1 beta (mlp path)
    bvec2 = b2 + ln1_b  # resid2 constant (residual path)

    xres = xb[Q0g : Q0g + NQ] + proj_b[None, :]

    mtiles = np.full((NQT, 128, KW), NEG, np.float32)
    for i in range(NQT):
        qg = Q0g + 128 * i
        valid = np.zeros((128, KW), np.float32)  # [q, k-in-window]
        for r in range(8):
            gr = row0 + 2 * i + r  # global image row of window row r
            if 0 <= gr < H_IMG:
                valid[:, 64 * r : 64 * (r + 1)] = (
                    mask[qg : qg + 128, 64 * gr : 64 * (gr + 1)] == 0
                )
        # coverage check: every allowed key lies inside the window
        full = mask[qg : qg + 128] == 0
        assert int(full.sum()) == int(valid.sum()), (core, i, "window coverage")
        # m[p, 128c+q] = 0 if valid[q, 128c+p] else NEG
        vT = valid.T.reshape(4, 128, 128).transpose(1, 0, 2).reshape(128, KW)
        mtiles[i][vT == 1.0] = 0.0

    rows = np.concatenate([ln1_g, ln2_g, ln2_b])[None, :]

    E128 = np.zeros((128, 128), np.float32)
    for k in range(4):
        E128[32 * k, 32 * k : 32 * k + 32] = 1.0

    return {
        "xT": _to_bf16(_fold128(slab.T)),
        "xres": _to_bf16(_fold128(xres)),
        "wqk": _to_bf16(_fold128(wqk.T)),
        "qkb": _fold128(qkb[:, None])[:, :, 0],
        "wv": _to_bf16(_fold128(wv_pad)),
        "wp": _to_bf16(_fold128(proj_w.T)),
        "w1": _to_bf16(_fold128(np.asarray(w1, np.float32).T)),
        "b1": _fold128(np.asarray(b1f, np.float32)[:, None])[:, :, 0],
        "w2": _to_bf16(_fold128(np.asarray(w2, np.float32).T)),
        "rows": np.ascontiguousarray(rows, dtype=np.float32),
        "bv2": _to_bf16(bvec2[None, :]),
        "mask": _to_bf16(np.ascontiguousarray(mtiles.transpose(1, 0, 2))),
        "E128": _to_bf16(E128),
    }


def kernel(**inputs):
    args = {k: np.asarray(v) for k, v in inputs.items()}
    in_maps = [
        _host_inputs(
            core,
            args["x"],
            np.asarray(args["mask"], dtype=np.float32),
            args["qkv_w"],
            args["qkv_b"],
            args["proj_w"],
            args["proj_b"],
            args["ln1_g"],
            args["ln1_b"],
            args["w1"],
            args["b1"],
            args["w2"],
            args["b2"],
            args["ln2_g"],
            args["ln2_b"],
        )
        for core in range(8)
    ]
    nc = _get_nc()
    res = run_bass_kernel_spmd(nc, in_maps, core_ids=list(range(8)))
    global _LAST_RESULT
    _LAST_RESULT = res
    out = np.zeros((B, N, C), np.float32)
    for core in range(8):
        b, half = core // 2, core % 2
        out[b, 1024 * half : 1024 * (half + 1)] = res.results[core]["out"]
    return out


# revision 17
# speedup vs baseline: 1.2741x; 1.2741x over previous
"""Trainium2 Bass kernel for nn_MixingBlock (local-window attention + MLP).

Sharding: 8 cores = (batch 0..3) x (token half 0..1); each core computes
1024 output tokens of one batch element on a zero-padded 22-image-row slab
(T=1408 tokens); no collectives (see kernel_baseline.py docstring).

v3 structure:
- additive mask (0 / -30000, bf16) folded into the score PSUM via
  identity-weight matmuls (one identity LDWEIGHTS per head-pair group);
- AV matmuls use V as the stationary operand (head-pairs, M=65: a-dims,
  b-dims, shared ones column) with the exp'd scores moving (N=256):
  128 MMs + 128 small LDWEIGHTS instead of 256 + 256 full-width ones, and
  the output lands directly in proj-ready (channels, queries) layout with
  softmax denominators as aligned extra rows -> no PE transposes after
  attention;
- denominators: 4 aligned row-copies into a memset-1.0 tile, one
  reciprocal, and an E128 selector matmul broadcasting each recip row
  across its 32-row head block; one tensor_tensor multiply per head slice
  normalizes while copying PSUM->SBUF bf16 (engine partition bases must
  be 32-aligned, DVE has no divide op);
- residual adds (x+proj, z1g+mlp2) folded into PSUM via identity matmuls;
- ln1_g applied once to z1 (w1 kept raw);
- K-projection PSUM->SBUF copies on vector, Q on scalar; Q projected only
  over the 1024 query tokens;
- inputs packed host-side into one multi-slice SBUF tile per tensor so
  each needs a single DMA; xres shipped as bf16; DMAs spread across the
  sync/gpsimd/scalar queues with bulk weights deferred past the first MM.
"""

import contextlib
import sys
import types

import ml_dtypes
import numpy as np

import concourse.bass as bass
import concourse.mybir as mybir
import concourse.tile as tile

# ---------------------------------------------------------------------------
# axon NTFF profile hook (lets run_bass_kernel_spmd(trace=True) work here)
# ---------------------------------------------------------------------------
if "antenv.axon_hooks" not in sys.modules:
    try:
        import antenv  # noqa: F401

        _hookmod = types.ModuleType("antenv.axon_hooks")
        _hookmod._hook = None
        _hookmod.set_axon_ntff_profile_hook = lambda h: setattr(_hookmod, "_hook", h)
        _hookmod.get_axon_ntff_profile_hook = lambda: _hookmod._hook
        sys.modules["antenv.axon_hooks"] = _hookmod
        try:
            from trn_agent_boot.trn_boot import _ntff_profile_via_ctypes

            _hookmod.set_axon_ntff_profile_hook(
                _ntff_profile_via_ctypes("/opt/axon/libaxon_pjrt.so")
            )
        except Exception:
            pass
    except Exception:
        pass

from concourse.bass_utils import run_bass_kernel_spmd  # noqa: E402
from concourse.masks import make_identity  # noqa: E402
from concourse.tile_rust import add_dep_helper  # noqa: E402

F32 = mybir.dt.float32
BF16 = mybir.dt.bfloat16
AF = mybir.ActivationFunctionType
ALU = mybir.AluOpType

# Problem constants
H_IMG, W_IMG = 32, 64
N = H_IMG * W_IMG  # 2048
C = 256
NH = 8
HD = 32
HIDDEN = 1024
SCALE = HD**-0.5
B = 4
LN_EPS = 1e-5

# Sharding constants
PAD_ROWS = 3
SLAB_ROWS = 16 + 2 * PAD_ROWS  # 22
T = SLAB_ROWS * W_IMG  # 1408
NQ = 1024
NQT = 8
KW = 512  # key window tokens per query tile
Q0 = PAD_ROWS * W_IMG  # 192: first query token within slab
NEG = -30000.0  # additive mask value for invalid keys

MAX_WAITS = 1


def _split_excess_waits(nc):
    """walrus accepts only MAX_WAITS sem-waits per instruction; move excess
    onto same-engine nops inserted immediately before the instruction."""
    for f in nc.m.functions:
        for bb in f.blocks:
            i = 0
            while i < len(bb.instructions):
                ins = bb.instructions[i]
                si = ins.sync_info
                if si is not None and si.on_wait and len(si.on_wait) > MAX_WAITS:
                    waits = list(si.on_wait)
                    extra, keep = waits[:-MAX_WAITS], waits[-MAX_WAITS:]
                    ins.sync_info = mybir.SyncInfo(
                        on_wait=keep, on_update=list(si.on_update)
                    )
                    nops = []
                    for j in range(0, len(extra), MAX_WAITS):
                        nop = nc.engines[ins.engine].nop().ins
                        cur = nc.cur_bb.bb
                        assert cur.instructions[-1] is nop
                        cur.instructions.pop()
                        nop.sync_info = mybir.SyncInfo(
                            on_wait=extra[j : j + MAX_WAITS], on_update=[]
                        )
                        nops.append(nop)
                    bb.instructions[i:i] = nops
                    i += len(nops)
                i += 1


def _build_nc():
    nc = bass.Bass("TRN2", target_bir_lowering=False, num_devices=8)

    d = {}
    d["xT"] = nc.dram_tensor("xT", [128, 2, T], BF16, kind="ExternalInput")
    d["xres"] = nc.dram_tensor("xres", [128, 8, C], BF16, kind="ExternalInput")
    d["wqk"] = nc.dram_tensor("wqk", [128, 2, 512], BF16, kind="ExternalInput")
    d["qkb"] = nc.dram_tensor("qkb", [128, 4], F32, kind="ExternalInput")
    d["wv"] = nc.dram_tensor("wv", [128, 2, 260], BF16, kind="ExternalInput")
    d["wp"] = nc.dram_tensor("wp", [128, 2, C], BF16, kind="ExternalInput")
    d["w1"] = nc.dram_tensor("w1", [128, 2, HIDDEN], BF16, kind="ExternalInput")
    d["b1"] = nc.dram_tensor("b1", [128, 8], F32, kind="ExternalInput")
    d["w2"] = nc.dram_tensor("w2", [128, 8, C], BF16, kind="ExternalInput")
    d["rows"] = nc.dram_tensor("rows", [1, 3 * C], F32, kind="ExternalInput")
    d["bv2"] = nc.dram_tensor("bv2", [1, C], BF16, kind="ExternalInput")
    d["mask"] = nc.dram_tensor("mask", [128, NQT, KW], BF16, kind="ExternalInput")
    d["E128"] = nc.dram_tensor("E128", [128, 128], BF16, kind="ExternalInput")
    d["out"] = nc.dram_tensor("out", [NQ, C], F32, kind="ExternalOutput")

    with tile.TileContext(nc) as tc:
        _emit(nc, tc, d)

    _split_excess_waits(nc)
    return nc


def _emit(nc, tc, d):
    ctx = contextlib.ExitStack()
    with ctx:
        const = ctx.enter_context(tc.tile_pool(name="const", bufs=1))
        big = ctx.enter_context(tc.tile_pool(name="big", bufs=1))
        ps_s = ctx.enter_context(tc.tile_pool(name="ps_s", bufs=2, space="PSUM"))
        ps_av = ctx.enter_context(tc.tile_pool(name="ps_av", bufs=2, space="PSUM"))
        ps = ctx.enter_context(tc.tile_pool(name="ps", bufs=2, space="PSUM"))
        work = ctx.enter_context(tc.tile_pool(name="work", bufs=4))
        small = ctx.enter_context(tc.tile_pool(name="small", bufs=4))

        late_dmas = []
        # -------- inputs: one multi-slice tile + one DMA per tensor ------
        xT = const.tile([128, 2, T], BF16, tag="xT", name="xT")
        wqk = const.tile([128, 2, 512], BF16, tag="wqk", name="wqk")
        wv = const.tile([128, 2, 260], BF16, tag="wv", name="wv")
        wp = const.tile([128, 2, C], BF16, tag="wp", name="wp")
        w1 = const.tile([128, 2, HIDDEN], BF16, tag="w1", name="w1s")
        qkb = const.tile([128, 4], F32, tag="qkb", name="qkb")
        b1 = const.tile([128, 8], F32, tag="b1", name="b1s")
        w2 = const.tile([128, 8, C], BF16, tag="w2", name="w2s")
        xres = const.tile([128, 8, C], BF16, tag="xres", name="xres")
        mask = const.tile([128, NQT, KW], BF16, tag="mask", name="mask")
        E128 = const.tile([128, 128], BF16, tag="E128", name="E128")
        denB = [const.tile([128, 128], F32, tag=f"denB{g}", name=f"denB{g}") for g in range(2)]

        # early DMAs: sync feeds the QK projection; gpsimd feeds masks
        # (needed from qt=0), wv and E128
        nc.sync.dma_start(out=xT[:], in_=d["xT"][:, :, :])
        nc.sync.dma_start(out=wqk[:], in_=d["wqk"][:, :, :])
        nc.sync.dma_start(out=qkb[:], in_=d["qkb"][:, :])
        nc.gpsimd.dma_start(out=wv[:], in_=d["wv"][:, :, :])
        nc.gpsimd.dma_start(out=mask[:], in_=d["mask"][:, :, :])
        nc.gpsimd.dma_start(out=E128[:], in_=d["E128"][:])
        for g in range(2):
            nc.vector.memset(denB[g][:], 1.0)
        # late DMAs (deferred until the first matmul has issued)
        late_dmas.append(nc.scalar.dma_start(out=wp[:], in_=d["wp"][:, :, :]).ins)
        late_dmas.append(nc.sync.dma_start(out=xres[:], in_=d["xres"][:, :, :]).ins)
        late_dmas.append(nc.sync.dma_start(out=w1[:], in_=d["w1"][:, :, :]).ins)
        late_dmas.append(nc.sync.dma_start(out=b1[:], in_=d["b1"][:, :]).ins)
        late_dmas.append(nc.sync.dma_start(out=w2[:], in_=d["w2"][:, :, :]).ins)
        bv2 = const.tile([1, C], BF16)
        late_dmas.append(nc.sync.dma_start(out=bv2[:], in_=d["bv2"][:]).ins)
        g1b = const.tile([128, C], F32)
        g2b = const.tile([128, C], F32)
        b2lb = const.tile([128, C], F32)
        nc.scalar.dma_start(
            out=g1b[:], in_=d["rows"][0:1, 0:C].to_broadcast((128, C))
        )
        nc.scalar.dma_start(
            out=g2b[:], in_=d["rows"][0:1, C : 2 * C].to_broadcast((128, C))
        )
        nc.scalar.dma_start(
            out=b2lb[:], in_=d["rows"][0:1, 2 * C : 3 * C].to_broadcast((128, C))
        )

        eps_t = const.tile([128, 1], F32)
        nc.vector.memset(eps_t[:], LN_EPS)
        ones_col = const.tile([1, 128], BF16)
        nc.vector.memset(ones_col[:], 1.0)
        ident = const.tile([128, 128], BF16)
        make_identity(nc, ident)

        # ---------------- phase 1: Q^T (scaled) and K^T ----------------
        # m pieces 0,1 = Q (only query tokens needed), 2,3 = K (full slab)
        qkT = [big.tile([128, T], BF16, tag=f"qkT{m}", name=f"qkT{m}") for m in range(4)]
        first_mm = [None]

        def qk_piece(m, lo, hi):
            for off in range(lo, hi, 512):
                w = min(512, hi - off)
                p = ps_s.tile([128, KW], F32, tag="s_ps", name="p_qk")
                for cc in range(2):
                    mm0 = nc.tensor.matmul(
                        p[:, :w],
                        wqk[:, cc, 128 * m : 128 * (m + 1)],
                        xT[:, cc, off : off + w],
                        start=(cc == 0),
                        stop=(cc == 1),
                    )
                    if first_mm[0] is None:
                        first_mm[0] = mm0
                        for dma in late_dmas:
                            add_dep_helper(dma, mm0.ins, sync=True,
                                           reason="defer bulk input DMA")
                if m < 2:
                    nc.scalar.activation(
                        out=qkT[m][:, off : off + w],
                        in_=p[:, :w],
                        func=AF.Identity,
                        bias=qkb[:, m : m + 1],
                        scale=1.0,
                    )
                else:
                    nc.vector.tensor_scalar_add(
                        out=qkT[m][:, off : off + w],
                        in0=p[:, :w],
                        scalar1=qkb[:, m : m + 1],
                    )

        for m in range(2):
            qk_piece(m, Q0, Q0 + NQ)
        for m in range(2, 4):
            qk_piece(m, 0, T)

        # ---------------- phase 2: V (token-major, ones columns) ----------
        vt = [big.tile([128, 260], BF16, tag=f"vt{i}", name=f"vt{i}") for i in range(T // 128)]
        for i in range(T // 128):
            p = ps_s.tile([128, 260], F32, tag="s_ps", name="p_v")
            for cc in range(2):
                nc.tensor.matmul(
                    p[:, :260],
                    xT[:, cc, 128 * i : 128 * (i + 1)],
                    wv[:, cc, :],
                    start=(cc == 0),
                    stop=(cc == 1),
                )
            nc.vector.tensor_copy(vt[i][:], p[:, :260])
            nc.gpsimd.memset(
                vt[i][:].rearrange("p (g e) -> p g e", g=4)[:, :, 64:65], 1.0
            )

        # ---------------- phase 3: attention ----------------
        attnT = [
            [
                big.tile([128, 128], BF16, tag=f"attnT{j}_{q}", name=f"attnT{j}_{q}")
                for q in range(NQT)
            ]
            for j in range(2)
        ]
        for qt in range(NQT):
            kw0 = 128 * qt
            for g in range(2):  # head-pair groups: pairs (2g, 2g+1)
                # av tile: [partition, pair-in-group, hi, query]
                av = ps_av.tile([128, 2, 2, 128], F32, tag="av", name=f"av{qt}_{g}")
                for pp in range(2):
                    pv = 2 * g + pp
                    h0 = 2 * pv
                    p_s = ps_s.tile([128, 2, KW], F32, tag="s_ps", name=f"p_s{qt}_{pv}")
                    for hi in range(2):
                        nc.tensor.matmul(
                            p_s[:, hi, :], ident[:], mask[:, qt, :],
                            start=True, stop=False,
                        )
                    for c in range(4):
                        for hi in range(2):
                            h = h0 + hi
                            ktile, koff = 2 + h // 4, (32 * h) % 128
                            qtile, qoff = h // 4, (32 * h) % 128
                            nc.tensor.matmul(
                                p_s[:, hi, 128 * c : 128 * (c + 1)],
                                qkT[ktile][
                                    koff : koff + 32,
                                    kw0 + 128 * c : kw0 + 128 * (c + 1),
                                ],
                                qkT[qtile][
                                    qoff : qoff + 32,
                                    Q0 + 128 * qt : Q0 + 128 * (qt + 1),
                                ],
                                start=False,
                                stop=(c == 3),
                                tile_position=(koff, 0),
                            )
                    pT = work.tile([128, 2, KW], BF16, tag="pT", name=f"pT{qt}_{pv}")
                    nc.scalar.activation(
                        out=pT[:], in_=p_s[:], func=AF.Exp, bias=0.0, scale=1.0
                    )
                    # AV: V pair stationary (M=65: a-dims, b-dims, ones col)
                    for c in range(4):
                        nc.tensor.matmul(
                            av[0:65, pp, :, :],
                            vt[qt + c][:, 65 * pv : 65 * pv + 65],
                            pT[:, :, 128 * c : 128 * (c + 1)],
                            start=(c == 0),
                            stop=(c == 3),
                        )
                # denominators: ones-rows -> 32k rows of denB, recip, then
                # R = E128^T @ rec expands row 32k to rows [32k,32k+32)
                for pp in range(2):
                    for hi in range(2):
                        k = 2 * pp + hi
                        nc.vector.tensor_copy(
                            denB[g][32 * k : 32 * k + 1, :], av[64:65, pp, hi, :]
                        )
                recB = small.tile([128, 128], BF16, tag="rec", name=f"rec{qt}_{g}")
                with nc.allow_low_precision(reason="bf16 recip of softmax denom"):
                    nc.vector.reciprocal(recB[:], denB[g][:])
                Rp = ps.tile([128, 128], F32, tag="mm", name=f"Rp{qt}_{g}")
                nc.tensor.matmul(Rp[:, :], E128[:], recB[:], start=True, stop=True)
                R = work.tile([128, 128], BF16, tag="Rs", name=f"Rs{qt}_{g}")
                nc.vector.tensor_copy(R[:], Rp[:, :])
                # normalize + copy to attnT (tile g holds heads 4g..4g+3)
                aT = attnT[g][qt]
                for pp in range(2):
                    for hi in range(2):
                        k = 2 * pp + hi
                        nc.vector.tensor_tensor(
                            out=aT[64 * pp + 32 * hi : 64 * pp + 32 * hi + 32, :],
                            in0=av[32 * hi : 32 * hi + 32, pp, hi, :],
                            in1=R[32 * k : 32 * k + 32, :],
                            op=ALU.mult,
                        )

        # ---------------- phase 4: proj + residual + LN1 ----------------
        z1g = [big.tile([128, C], BF16, tag=f"z1g{i}", name=f"z1g_{i}") for i in range(8)]
        for t in range(8):
            p_p = ps.tile([128, C], F32, tag="mm", name="p_p")
            nc.tensor.matmul(p_p[:, :C], ident[:], xres[:, t, :], start=True, stop=False)
            for cc in range(2):
                nc.tensor.matmul(
                    p_p[:, :C],
                    attnT[cc][t][:],
                    wp[:, cc, :],
                    start=False,
                    stop=(cc == 1),
                )
            stats = small.tile([128, 6], F32, tag="stats")
            nc.vector.bn_stats(out=stats[:], in_=p_p[:, :C])
            mv = small.tile([128, 2], F32, tag="mv")
            nc.vector.bn_aggr(out=mv[:], in_=stats[:])
            lnv = small.tile([128, 1], F32, tag="lnv")
            nc.scalar.activation(
                out=lnv[:], in_=mv[:, 1:2], func=AF.Ln, bias=eps_t[:], scale=1.0
            )
            rstd = small.tile([128, 1], F32, tag="rstd")
            nc.scalar.activation(
                out=rstd[:], in_=lnv[:], func=AF.Exp, bias=0.0, scale=-0.5
            )
            zc = work.tile([128, C], F32, tag="zc")
            nc.vector.scalar_tensor_tensor(
                out=zc[:],
                in0=p_p[:, :C],
                scalar=mv[:, 0:1],
                in1=g1b[:],
                op0=ALU.subtract,
                op1=ALU.mult,
            )
            nc.vector.tensor_scalar_mul(out=z1g[t][:], in0=zc[:], scalar1=rstd[:])

        # ---------------- phase 4b: z1g^T (PE transpose) ----------------
        z1T = [
            [
                big.tile([128, 512], BF16, tag=f"z1T{j}_{p}", name=f"z1T{j}_{p}")
                for p in range(2)
            ]
            for j in range(2)
        ]
        for t in range(8):
            for j in range(2):
                p_t = ps.tile([128, 128], BF16, tag="mm", name="p_t")
                nc.tensor.transpose(
                    p_t[:, :128], z1g[t][:, 128 * j : 128 * (j + 1)], ident[:]
                )
                nc.vector.tensor_copy(
                    z1T[j][t // 4][:, 128 * (t % 4) : 128 * (t % 4 + 1)],
                    p_t[:, :128],
                )

        # ---------------- phase 5: mlp1 + gelu (hidden-major) ------------
        hT = [
            [
                big.tile([128, 512], BF16, tag=f"hT{i}_{p}", name=f"hT{i}_{p}")
                for p in range(2)
            ]
            for i in range(8)
        ]
        for piece in range(2):
            for hc in range(8):
                p_h = ps_s.tile([128, KW], F32, tag="s_ps", name="p_h")
                for cc in range(2):
                    nc.tensor.matmul(
                        p_h[:, :],
                        w1[:, cc, 128 * hc : 128 * (hc + 1)],
                        z1T[cc][piece][:],
                        start=(cc == 0),
                        stop=(cc == 1),
                    )
                nc.scalar.activation(
                    out=hT[hc][piece][:],
                    in_=p_h[:, :],
                    func=AF.Gelu,
                    bias=b1[:, hc : hc + 1],
                    scale=1.0,
                )

        # ---------------- phase 6: mlp2 + resid2 + LN2 + out -------------
        for t in range(8):
            p_m = ps.tile([128, C], F32, tag="mm", name="p_m")
            nc.tensor.matmul(p_m[:, :C], ones_col[:], bv2[:], start=True, stop=False)
            nc.tensor.matmul(p_m[:, :C], ident[:], z1g[t][:], start=False, stop=False)
            for hc in range(8):
                nc.tensor.matmul(
                    p_m[:, :C],
                    hT[hc][t // 4][:, 128 * (t % 4) : 128 * (t % 4 + 1)],
                    w2[:, hc, :],
                    start=False,
                    stop=(hc == 7),
                )
            stats = small.tile([128, 6], F32, tag="stats2")
            nc.vector.bn_stats(out=stats[:], in_=p_m[:, :C])
            mv = small.tile([128, 2], F32, tag="mv2")
            nc.vector.bn_aggr(out=mv[:], in_=stats[:])
            lnv = small.tile([128, 1], F32, tag="lnv2")
            nc.scalar.activation(
                out=lnv[:], in_=mv[:, 1:2], func=AF.Ln, bias=eps_t[:], scale=1.0
            )
            rstd = small.tile([128, 1], F32, tag="rstd2")
            nc.scalar.activation(
                out=rstd[:], in_=lnv[:], func=AF.Exp, bias=0.0, scale=-0.5
            )
            zc = work.tile([128, C], F32, tag="zc2")
            nc.vector.scalar_tensor_tensor(
                out=zc[:],
                in0=p_m[:, :C],
                scalar=mv[:, 0:1],
                in1=g2b[:],
                op0=ALU.subtract,
                op1=ALU.mult,
            )
            zz = work.tile([128, C], F32, tag="zz")
            nc.vector.tensor_scalar_mul(out=zz[:], in0=zc[:], scalar1=rstd[:])
            o = work.tile([128, C], F32, tag="o")
            nc.vector.tensor_add(o[:], zz[:], b2lb[:])
            nc.sync.dma_start(out=d["out"][128 * t : 128 * (t + 1), :], in_=o[:])


_NC_CACHE = None
_LAST_RESULT = None


def _get_nc():
    global _NC_CACHE
    if _NC_CACHE is None:
        _NC_CACHE = _build_nc()
    return _NC_CACHE


def _to_bf16(a):
    return np.ascontiguousarray(np.asarray(a, dtype=np.float32)).astype(
        ml_dtypes.bfloat16
    )


def _fold128(a):
    """[128*k, cols...] -> [128, k, cols...] (partition-major packing)."""
    a = np.asarray(a, dtype=np.float32)
    k = a.shape[0] // 128
    return np.ascontiguousarray(
        a.reshape(k, 128, *a.shape[1:]).transpose(1, 0, *range(2, a.ndim + 1))
    )


def _host_inputs(core, x, mask, qkv_w, qkv_b, proj_w, proj_b, ln1_g, ln1_b, w1,
                 b1, w2, b2, ln2_g, ln2_b):
    b = core // 2
    half = core % 2
    row0 = 16 * half - PAD_ROWS  # slab start image row (may be negative)
    S0 = row0 * W_IMG  # slab start token
    Q0g = 1024 * half  # first query token (global)

    xb = np.asarray(x[b], dtype=np.float32)  # [N, C]
    slab = np.zeros((T, C), np.float32)
    g_lo, g_hi = max(0, S0), min(N, S0 + T)
    slab[g_lo - S0 : g_hi - S0] = xb[g_lo:g_hi]

    wqk = np.concatenate([qkv_w[:C] * SCALE, qkv_w[C : 2 * C]], axis=0)  # [512,C]
    qkb = np.concatenate([qkv_b[:C] * SCALE, qkv_b[C : 2 * C]])  # [512]
    wv = qkv_w[2 * C :]  # [256, 256]
    vb = qkv_b[2 * C :]
    assert np.abs(vb).max() == 0.0, "nonzero v bias not folded"
    wv_pad = np.zeros((C, 260), np.float32)
    for p in range(4):
        wv_pad[:, 65 * p : 65 * p + 32] = wv[64 * p : 64 * p + 32].T
        wv_pad[:, 65 * p + 32 : 65 * p + 64] = wv[64 * p + 32 : 64 * p + 64].T

    b1f = b1 + w1 @ ln1_b  # fold ln1 beta (mlp path)
    bvec2 = b2 + ln1_b  # resid2 constant (residual path)

    xres = xb[Q0g : Q0g + NQ] + proj_b[None, :]

    mtiles = np.full((NQT, 128, KW), NEG, np.float32)
    for i in range(NQT):
        qg = Q0g + 128 * i
        valid = np.zeros((128, KW), np.float32)  # [q, k-in-window]
        for r in range(8):
            gr = row0 + 2 * i + r  # global image row of window row r
            if 0 <= gr < H_IMG:
                valid[:, 64 * r : 64 * (r + 1)] = (
                    mask[qg : qg + 128, 64 * gr : 64 * (gr + 1)] == 0
                )
        # coverage check: every allowed key lies inside the window
        full = mask[qg : qg + 128] == 0
        assert int(full.sum()) == int(valid.sum()), (core, i, "window coverage")
        # m[p, 128c+q] = 0 if valid[q, 128c+p] else NEG
        vT = valid.T.reshape(4, 128, 128).transpose(1, 0, 2).reshape(128, KW)
        mtiles[i][vT == 1.0] = 0.0

    rows = np.concatenate([ln1_g, ln2_g, ln2_b])[None, :]

    E128 = np.zeros((128, 128), np.float32)
    for k in range(4):
        E128[32 * k, 32 * k : 32 * k + 32] = 1.0

    return {
        "xT": _to_bf16(_fold128(slab.T)),
        "xres": _to_bf16(_fold128(xres)),
        "wqk": _to_bf16(_fold128(wqk.T)),
        "qkb": _fold128(qkb[:, None])[:, :, 0],
        "wv": _to_bf16(_fold128(wv_pad)),
        "wp": _to_bf16(_fold128(proj_w.T)),
        "w1": _to_bf16(_fold128(np.asarray(w1, np.float32).T)),
        "b1": _fold128(np.asarray(b1f, np.float32)[:, None])[:, :, 0],
        "w2": _to_bf16(_fold128(np.asarray(w2, np.float32).T)),
        "rows": np.ascontiguousarray(rows, dtype=np.float32),
        "bv2": _to_bf16(bvec2[None, :]),
        "mask": _to_bf16(np.ascontiguousarray(mtiles.transpose(1, 0, 2))),
        "E128": _to_bf16(E128),
    }


def kernel(**inputs):
    args = {k: np.asarray(v) for k, v in inputs.items()}
    in_maps = [
        _host_inputs(
            core,
            args["x"],
            np.asarray(args["mask"], dtype=np.float32),
            args["qkv_w"],
            args["qkv_b"],
            args["proj_w"],
            args["proj_b"],
            args["ln1_g"],
            args["ln1_b"],
            args["w1"],
            args["b1"],
            args["w2"],
            args["b2"],
            args["ln2_g"],
            args["ln2_b"],
        )
        for core in range(8)
    ]
    nc = _get_nc()
    res = run_bass_kernel_spmd(nc, in_maps, core_ids=list(range(8)))
    global _LAST_RESULT
    _LAST_RESULT = res
    out = np.zeros((B, N, C), np.float32)
    for core in range(8):
        b, half = core // 2, core % 2
        out[b, 1024 * half : 1024 * (half + 1)] = res.results[core]["out"]
    return out
